# revision 45
# baseline (speedup 1.0000x reference)
"""ColBERT pairwise scoring kernel for 8x TRN2 NeuronCores.

Computation (see problem reference):
    qn = l2norm(q, axis=-1); kn = l2norm(k, axis=-1)
    S[b,o,i,j] = qn[b,i,:]·kn[o,j,:], masked positions -> -inf
    s[b,o] = sum_i logsumexp_j(ALPHA*S)/ALPHA, nonfinite -> 0
    out = s / (sqrt(Lq*Lk)+1e-6) * min(exp(logit_scale), 100)

Key observations exploited here:
  * Any batch row b with >= 1 masked query token is exactly 0 in the
    reference output (the -inf from that token survives the sum over Lq and
    is then zeroed).  Only rows with a fully-unmasked query need computing.
    The host packs those rows (up to QB_CAP=8 of them per pass) into a
    [256, D] tile; if more survive, the same program is run multiple times.
  * bf16 matmuls run 4x faster than fp32 on the PE (1 cycle/col vs 4), and
    |S| <= 1 with unit-norm rows, so bf16 inputs keep the overall relative
    error ~1e-3, far inside the 2e-2 gate.  No max-subtraction is needed for
    the logsumexp since |ALPHA*S| <= 12.

Sharding: candidate axis O split across 8 cores (OL=16 o's per core), packed
q replicated.  Per-core pipeline (all bf16 except PSUM):
    k arrives paired ([2048, 256] view of [4096, 128] so DMA descriptors are
    512B) -> row norms on DVE (scalar_tensor_tensor square+accumulate) ->
    rk = ALPHA/||k|| via exp(-0.5*ln) on ACT (stays in one activation table)
    -> DVE scales k rows by rk -> PE transposes to kt[d, j] -> main matmul
    kt_chunk^T @ qt -> ACT exp over [128,1024] groups -> indicator-column
    matmul accumulates per-o sums into two half-size PSUM tiles (the first
    half's Ln/reduce/store overlaps the second half's groups) ->
    Ln(sum - n_masked) -> sum over Lq -> DMA out.
The whole prep/main pipeline is software-scheduled around the in-order
engine queues (4-deep OOO window): quad-granular k prep interleaves with the
main-loop groups so DVE (the busiest engine) stays saturated.
Masked k rows are zeroed on the host (exp contributes exactly 1.0 there) and
the per-o masked count is subtracted inside the final Ln's bias.
"""

import math
import sys
from contextlib import ExitStack

import ml_dtypes
import numpy as np

BF16NP = ml_dtypes.bfloat16

for _p in ("/opt/trn_rl_repo",):
    if _p not in sys.path:
        sys.path.insert(0, _p)

import concourse.bass as bass
import concourse.bacc as bacc
import concourse.tile as tile
from concourse import mybir
from concourse.bass_utils import run_bass_kernel_spmd

ALPHA = 12.0
B, Lq, O, Lk, D = 64, 32, 128, 256, 128
NCORES = 8
OL = O // NCORES          # candidates per core = 16
KR = OL * Lk              # k rows per core = 4096
NKP = KR // 256           # paired k chunks per core = 16
QB_CAP = 8                # max surviving batch rows per pass
BIP = QB_CAP * Lq         # packed query rows = 256

F32 = mybir.dt.float32
BF16 = mybir.dt.bfloat16
AF = mybir.ActivationFunctionType
OP = mybir.AluOpType


def emit_kernel(ctx, tc, q_d, k_d, cd_d, out_d):
    nc = tc.nc
    NCH = KR // 128           # 128-row j chunks = 32
    NG = NCH // 4             # exp groups of 4 chunks = 8
    NQD = 4                   # k arrives in 4 quarter DMAs
    QP = NKP // NQD           # paired chunks per quarter = 4

    sing = ctx.enter_context(tc.tile_pool(name="sing", bufs=1))
    kscp = ctx.enter_context(tc.tile_pool(name="ksc", bufs=4))
    sqp = ctx.enter_context(tc.tile_pool(name="sq", bufs=6))
    pmt = ctx.enter_context(tc.tile_pool(name="pmt", bufs=2, space="PSUM"))
    pm = ctx.enter_context(tc.tile_pool(name="pm", bufs=2, space="PSUM"))
    plse = ctx.enter_context(tc.tile_pool(name="plse", bufs=2, space="PSUM"))
    etp = ctx.enter_context(tc.tile_pool(name="et", bufs=2))

    kin = sing.tile([128, NKP, 256], BF16)   # paired natural k
    qin = sing.tile([128, 2, 128], BF16)     # natural packed q
    kt = sing.tile([128, KR], BF16)          # scaled transposed k [d, j]
    qt = sing.tile([128, BIP], BF16)         # normalized transposed q [d, bi]
    nk = sing.tile([128, 2 * NKP], F32)      # k row squared norms
    rk = sing.tile([128, 2 * NKP], F32)      # ALPHA / ||k row||
    nq = sing.tile([128, 2], F32)
    rq = sing.tile([128, 2], F32)
    cdat = sing.tile([128, 128 + OL * OL + 2], BF16)  # id128 ++ ecols ++ negnm
    id128 = cdat[:, 0:128]
    ecols = cdat[:, 128:128 + OL * OL]
    negnm = sing.tile([128, 2], F32)   # col h: -n_masked for o in half h
    loglse = sing.tile([OL // 2, BIP], F32)
    sres = sing.tile([OL // 2, 2 * QB_CAP], F32)
    bias_eps = sing.tile([128, 1], F32)      # 1e-30, keeps Ln input nonzero
    bias_lna = sing.tile([128, 1], F32)      # ln(ALPHA), folds alpha into rk
    nc.vector.memset(bias_eps, 1e-30)
    nc.vector.memset(bias_lna, math.log(ALPHA))

    # ---- inputs in: q first (its prep finishes before k lands), then k
    #      quarters, all on SP; constants ride the ACT queue ----
    kre = k_d.rearrange("(c p) e -> p c e", p=128)
    nc.sync.dma_start(out=qin, in_=q_d.rearrange("(c p) d -> p c d", p=128))
    for qd in range(NQD):
        nc.sync.dma_start(out=kin[:, qd * QP:(qd + 1) * QP, :],
                          in_=kre[:, qd * QP:(qd + 1) * QP, :])
    nc.scalar.dma_start(out=cdat, in_=cd_d)

    # ---- helpers ----
    def emit_knorms(p0, p1):
        for c in range(p0, p1):
            for s in range(2):
                sq = sqp.tile([128, 128], BF16, tag="sq")
                nc.vector.scalar_tensor_tensor(
                    out=sq, in0=kin[:, c, s * 128:(s + 1) * 128], scalar=1.0,
                    in1=kin[:, c, s * 128:(s + 1) * 128],
                    op0=OP.mult, op1=OP.mult,
                    accum_out=nk[:, 2 * c + s:2 * c + s + 1])

    def emit_rk(c0, c1):
        # rk = ALPHA * exp(-0.5 * ln(n2)): stays within the ln/exp ACT table
        nc.scalar.activation(out=nk[:, 2 * c0:2 * c1], in_=nk[:, 2 * c0:2 * c1],
                             func=AF.Ln, bias=bias_eps[:, 0:1], scale=1.0)
        nc.scalar.activation(out=rk[:, 2 * c0:2 * c1], in_=nk[:, 2 * c0:2 * c1],
                             func=AF.Exp, bias=bias_lna[:, 0:1], scale=-0.5)

    # ---- q prep first (qin lands before k), then per-quad norm->rk->prep
    #      so quad 0 feeds the first main matmul as early as possible ----
    for c in range(2):
        sq = sqp.tile([128, 128], BF16, tag="sq")
        nc.vector.scalar_tensor_tensor(
            out=sq, in0=qin[:, c, :], scalar=1.0, in1=qin[:, c, :],
            op0=OP.mult, op1=OP.mult, accum_out=nq[:, c:c + 1])
    nc.scalar.activation(out=nq, in_=nq, func=AF.Ln,
                         bias=bias_eps[:, 0:1], scale=1.0)
    nc.scalar.activation(out=rq, in_=nq, func=AF.Exp, bias=0.0, scale=-0.5)
    qs = kscp.tile([128, 2, 128], BF16, tag="qs")
    pq = pmt.tile([128, 1024], BF16, tag="pt")
    for c in range(2):
        nc.vector.tensor_scalar(out=qs[:, c, :], in0=qin[:, c, :],
                                scalar1=rq[:, c:c + 1], scalar2=None,
                                op0=OP.mult)
        nc.tensor.transpose(out=pq[:, c * 128:(c + 1) * 128],
                            in_=qs[:, c, :], identity=id128)
    nc.vector.tensor_copy(out=qt, in_=pq[:, 0:256])

    # k prep: per quad qd (pairs 2qd, 2qd+1 -> kt cols qd*512..) norms, rk,
    # scale, transpose into an "oct" [128, 1024] PSUM tile shared by quad
    # pairs; one copy per oct (ACT pre-loop where it idles, DVE mid-loop).
    octs = {}

    def emit_kprep_quad(qd, copy_eng, single=False):
        # single=True: quad gets its own [128, 512] copy right away (lower
        # latency for the first mains); else pairs share one 1024-wide copy.
        if single or qd % 2 == 0:
            ot = pmt.tile([128, 1024], BF16, tag="pt")
            octs[qd] = ot
            off = 0
        else:
            off = 512
        pt = octs[qd if (single or qd % 2 == 0) else qd - 1]
        ks = kscp.tile([128, 4, 128], BF16, tag="ks")
        for h in range(4):
            c, s = 2 * qd + h // 2, h % 2
            nc.vector.tensor_scalar(
                out=ks[:, h, :], in0=kin[:, c, s * 128:(s + 1) * 128],
                scalar1=rk[:, 2 * c + s:2 * c + s + 1], scalar2=None,
                op0=OP.mult)
            nc.tensor.transpose(
                out=pt[:, off + h * 128:off + (h + 1) * 128],
                in_=ks[:, h, :], identity=id128)
        done = single or qd % 2 == 1
        if done:
            src = pt[:, 0:512] if single else pt
            base = qd if single else qd - 1
            dst = kt[:, base * 512:(base + (1 if single else 2)) * 512]
            octs.pop(qd if single else qd - 1, None)
            if copy_eng == "act":
                nc.scalar.copy(out=dst, in_=src)
            else:
                nc.vector.tensor_copy(out=dst, in_=src)

    emit_knorms(0, 2)
    emit_rk(0, 2)
    emit_kprep_quad(0, "act", single=True)
    emit_knorms(2, 4)
    emit_rk(2, 4)
    emit_kprep_quad(1, "act", single=True)

    # ---- main loop, software-pipelined:
    #        exp(g) ; m(g+1) ; prep quad g+4 ; reduce(g)
    #      lse is split in two halves so the first half's tail overlaps ----
    lseA = plse.tile([OL, BIP], F32, tag="lse")
    lseB = plse.tile([OL, BIP], F32, tag="lse")

    def emit_tail(lse, h):
        # half h holds o = 8h..8h+7 in lse rows 0..7 (host remaps ecols)
        HO = OL // 2
        nc.scalar.activation(out=loglse, in_=lse[0:HO, :],
                             func=AF.Ln, bias=negnm[0:HO, h:h + 1], scale=1.0)
        nc.vector.tensor_reduce(
            out=sres[:, h * QB_CAP:(h + 1) * QB_CAP],
            in_=loglse.rearrange("p (b i) -> p b i", i=Lq),
            axis=mybir.AxisListType.X, op=OP.add)
        nc.sync.dma_start(out=out_d[h * HO:(h + 1) * HO, :],
                          in_=sres[:, h * QB_CAP:(h + 1) * QB_CAP])

    def emit_mains(g):
        T = pm.tile([128, 4 * BIP], F32, tag="mm")
        for s in range(4):
            ch = 4 * g + s
            nc.tensor.matmul(
                out=T[:, s * BIP:(s + 1) * BIP],
                lhsT=kt[:, ch * 128:(ch + 1) * 128],
                rhs=qt, start=True, stop=True)
        return T

    Ts = {0: emit_mains(0)}
    emit_knorms(4, 8)
    nc.vector.tensor_copy(out=negnm, in_=cdat[:, 384:386])  # bf16 -> f32

    for g in range(NG):
        e = etp.tile([128, 4 * BIP], BF16, tag="e")
        nc.scalar.activation(out=e, in_=Ts.pop(g), func=AF.Exp,
                             bias=0.0, scale=1.0)
        if g + 1 < NG:
            Ts[g + 1] = emit_mains(g + 1)
        lse = lseA if g < NG // 2 else lseB
        for s in range(4):
            o = (4 * g + s) // 2
            nc.tensor.matmul(
                out=lse[0:OL, :],
                lhsT=ecols[:, o * OL:(o + 1) * OL],
                rhs=e[:, s * BIP:(s + 1) * BIP],
                start=(g % (NG // 2) == 0 and s == 0),
                stop=(g % (NG // 2) == NG // 2 - 1 and s == 3))
        if g == 0:
            emit_rk(4, 8)
            emit_kprep_quad(2, "dve", single=True)
            emit_kprep_quad(3, "dve", single=True)
        elif g == 1:
            emit_knorms(8, 12)
            emit_rk(8, 12)
        elif g == 2:
            emit_kprep_quad(4, "dve", single=True)
            emit_kprep_quad(5, "dve", single=True)
        elif g == 3:
            emit_knorms(12, NKP)
            emit_rk(12, NKP)
            emit_kprep_quad(6, "dve", single=True)
        elif g == 4:
            emit_kprep_quad(7, "dve", single=True)
        if g == NG // 2 - 1:
            emit_tail(lseA, 0)
    emit_tail(lseB, 1)


def _patch_act_tables():
    """Make Bacc's act-table-load inserter pick one table serving both Exp
    and Ln (e.g. natural_log_exp_and_others) instead of thrashing between
    single-function tables: blank out any exp/ln table that doesn't contain
    both.  Entry positions (= act_func_set_id) are preserved."""
    import concourse.bacc as bacc_mod
    from concourse.hw_specs import get_activation_tables as gat
    if getattr(bacc_mod, "_act_tables_patched", False):
        return
    exp, ln = AF.Exp, AF.Ln

    def patched(arch):
        tabs = gat(arch)
        out = {}
        for name, s in tabs.items():
            has_e, has_l = exp in s, ln in s
            if (has_e or has_l) and not (has_e and has_l):
                s = s - {exp, ln}
            out[name] = s
        return out

    bacc_mod.get_activation_tables = patched
    bacc_mod._act_tables_patched = True


def build_program():
    _patch_act_tables()
    nc = bacc.Bacc("TRN2", target_bir_lowering=False, debug=False,
                   enable_asserts=False, num_devices=NCORES)
    q_d = nc.dram_tensor("q_in", [BIP, D], BF16, kind="ExternalInput").ap()
    k_d = nc.dram_tensor("k_in", [KR // 2, 2 * D], BF16, kind="ExternalInput").ap()
    cd_d = nc.dram_tensor("cdat", [128, 128 + OL * OL + 2], BF16,
                          kind="ExternalInput").ap()
    out_d = nc.dram_tensor("outp", [OL, QB_CAP], F32, kind="ExternalOutput").ap()

    with tile.TileContext(nc) as tc, ExitStack() as ctx:
        emit_kernel(ctx, tc, q_d, k_d, cd_d, out_d)
    nc.compile()
    return nc


def make_in_maps(q, k, q_mask, k_mask, OL_=None, ncores=NCORES):
    """Host-side shard prep.  Returns (passes, groups): passes is a list of
    per-core input-dict lists (one entry per device pass), groups the list of
    surviving batch indices handled by each pass."""
    q = np.asarray(q, dtype=np.float32)
    k = np.asarray(k, dtype=np.float32)
    q_mask = np.asarray(q_mask).astype(bool)
    k_mask = np.asarray(k_mask).astype(bool)

    surv = np.nonzero(~q_mask.any(axis=1))[0]
    groups = [surv[i:i + QB_CAP] for i in range(0, len(surv), QB_CAP)]
    if not groups:
        groups = [np.zeros((0,), dtype=np.int64)]

    kz = k.copy()
    kz[k_mask] = 0.0
    nmask = k_mask.sum(axis=1).astype(np.float32)           # [O]
    cdat0 = np.zeros((128, 128 + OL * OL + 2), dtype=BF16NP)
    cdat0[:, 0:128] = np.eye(128, dtype=BF16NP)
    for o in range(OL):
        # candidate o accumulates into lse row o % 8 of its half's tile
        cdat0[:, 128 + o * OL + (o % (OL // 2))] = 1.0

    core_static = []
    for c in range(ncores):
        osl = slice(c * OL, (c + 1) * OL)
        cdat = cdat0.copy()
        # negnm: exact small integers, representable in bf16; col h covers
        # the o's of half h
        nm = -nmask[osl]
        cdat[:OL // 2, -2] = nm[:OL // 2].astype(BF16NP)
        cdat[:OL // 2, -1] = nm[OL // 2:].astype(BF16NP)
        core_static.append({
            "k_in": np.ascontiguousarray(
                kz[osl].reshape(KR // 2, 2 * D).astype(BF16NP)),
            "cdat": cdat,
        })

    passes = []
    for g in groups:
        qp = np.zeros((BIP, D), dtype=np.float32)
        if len(g):
            qp[:len(g) * Lq] = q[g].reshape(len(g) * Lq, D)
        qp = qp.astype(BF16NP)
        in_maps = []
        for c in range(ncores):
            m = dict(core_static[c])
            m["q_in"] = qp
            in_maps.append(m)
        passes.append(in_maps)
    return passes, groups


def postprocess(per_pass_outs, groups, k_mask, logit_scale, ncores=NCORES):
    """Scatter per-pass [OL, QB_CAP] core results into the [B, O] output."""
    out = np.zeros((B, O), dtype=np.float32)
    coef = min(math.exp(float(logit_scale)), 100.0) / (
        ALPHA * (math.sqrt(Lq * Lk) + 1e-06))
    for outs, g in zip(per_pass_outs, groups):
        nb = len(g)
        if nb == 0:
            continue
        for c in range(ncores):
            blk = np.asarray(outs[c])[:, :nb].T * np.float32(coef)  # [nb, OL]
            out[np.asarray(g), c * OL:(c + 1) * OL] = blk
    out[:, np.asarray(k_mask).astype(bool).all(axis=1)] = 0.0
    return np.where(np.isfinite(out), out, 0.0).astype(np.float32)


_CACHED_NC = None


def kernel(q, k, q_mask, k_mask, logit_scale):
    global _CACHED_NC
    if _CACHED_NC is None:
        _CACHED_NC = build_program()
    passes, groups = make_in_maps(q, k, q_mask, k_mask)
    per_pass_outs = []
    for in_maps in passes:
        res = run_bass_kernel_spmd(_CACHED_NC, in_maps, list(range(NCORES)))
        per_pass_outs.append(
            [np.asarray(res.results[c]["outp"]) for c in range(NCORES)])
    return postprocess(per_pass_outs, groups, k_mask, logit_scale)
